# revision 45
# baseline (speedup 1.0000x reference)
"""CenterLoss kernel for Trainium2 (8 NeuronCores, data-parallel over batch).

reference:  mean(clip(rowsum((x - labels @ centers)^2), 1e-12, 1e12))
labels are exact one-hot rows, so labels @ centers is an embedding gather:
    idx[b]  = max_index(labels[b, :])           (DVE max_index, query = 1.0)
    c[b]    = centers[idx[b], :]                (indirect DMA row gather)
    ps[b]   = rowsum((x[b] - c[b])^2)           (DVE sub, ACT square+f32 accum)

All three input streams are cast to bf16 at shard time (the tolerance for
this loss is 2e-2; bf16 keeps the scalar error ~1e-4), halving HBM traffic
to ~9.6MB/core. One-hot labels are exact in bf16, per-sample sums
accumulate in f32 on the ACT engine.

Schedule: every load rides the single SWDGE queue FIFO (two queues running
concurrently drop aggregate DMA from ~430 to ~320 GB/s, so one queue only):
labels (split so FIND0 unblocks early) -> row gathers as the index chain
delivers offsets -> x chunks, with the last tile split into quarters so the
sub/square tail after the final chunk is short. Per-core output is a
[128, 11] tile of per-sample (partial) sums; the host merges the last
tile's quarters, applies the clip (never binding for this data, but exact)
and takes the mean.
"""

import numpy as np
import ml_dtypes

import concourse.bacc as bacc
import concourse.bass as bass
import concourse.mybir as mybir
from concourse.tile import TileContext
from concourse.bass_utils import run_bass_kernel_spmd

F32 = mybir.dt.float32
BF16 = mybir.dt.bfloat16
U32 = mybir.dt.uint32
NP_BF16 = ml_dtypes.bfloat16

NCORES = 8
B = 8192          # full batch
C = 751           # num classes
D = 2048          # feature dim
BS = B // NCORES  # batch per core = 1024
P = 128           # partitions
NT = BS // P      # batch tiles per core = 8
NQ = 4            # last tile split into quarters for a short tail
NACC = NT - 1 + NQ

CLIP_LO, CLIP_HI = 1e-12, 1e12


def build_nc():
    nc = bacc.Bacc(
        "TRN2",
        target_bir_lowering=False,
        debug=False,
        num_devices=NCORES,
    )
    x = nc.dram_tensor("x", [BS, D], BF16, kind="ExternalInput")
    labels = nc.dram_tensor("labels", [BS, C], BF16, kind="ExternalInput")
    centers = nc.dram_tensor("centers", [C, D], BF16, kind="ExternalInput")
    out = nc.dram_tensor("out", [P, NACC], F32, kind="ExternalOutput")

    with TileContext(nc) as tc:
        with tc.tile_pool(name="big", bufs=1) as pool:
            ones = pool.tile([P, 8], BF16)
            idxs = pool.tile([P, NT, 8], U32)
            acc = pool.tile([P, NACC], F32)
            dif_a = pool.tile([P, D], BF16)
            dif_b = pool.tile([P, D], BF16)
            dif_c = pool.tile([P, D], BF16)
            dif_d = pool.tile([P, D], BF16)
            dsq = pool.tile([P, D], BF16)
            tjunk = pool.tile([P, D], BF16)
            lbig = pool.tile([P, NT, C], BF16)
            xbig = pool.tile([P, NT, D], BF16)
            ctile = pool.tile([P, NT, D], BF16)

            nc.vector.memset(ones[:], 1.0)

            labels_r = labels.rearrange("(n p) c -> p n c", p=P)
            x_r = x.rearrange("(n p) d -> p n d", p=P)

            # labels first; the single-tile first chunk rides the sync HWDGE
            # ring (shorter completion latency, warms the ring for the out
            # store) so FIND0 and the first gather unblock early
            # all labels chunks on the sync ring: the SWDGE queue is idle
            # during the labels phase anyway (its first item waits FIND0),
            # and HWDGE completion latency is ~2-3us shorter, so the FIND
            # chain never stalls on a label chunk's semaphore
            for lo, hi in ((0, 1), (1, 4), (4, 8)):
                nc.sync.dma_start(
                    out=lbig[:, lo:hi, :], in_=labels_r[:, lo:hi, :]
                )

            # Tile 7's x rides the idle sync ring at t0, so the SWDGE queue
            # ends with tile 6 split into quarters: the tail after the last
            # queue item is one quarter-sub + quarter-square instead of a
            # full tile. Gather order puts G7 mid-stream and G6 before the
            # quarters so both late tiles' gathers land before their x.
            Q = D // NQ
            nc.sync.dma_start(
                out=xbig[:, NT - 1, :], in_=x_r[:, NT - 1, :]
            )

            for n in range(NT):
                nc.vector.max_index(
                    out=idxs[:, n, :], in_max=ones[:], in_values=lbig[:, n, :]
                )

            def gather(n):
                nc.gpsimd.indirect_dma_start(
                    out=ctile[:, n, :],
                    out_offset=None,
                    in_=centers[:],
                    in_offset=bass.IndirectOffsetOnAxis(
                        ap=idxs[:, n, 0:1], axis=0
                    ),
                )

            def load_xf(lo, hi):
                nc.gpsimd.dma_start(out=xbig[:, lo:hi, :], in_=x_r[:, lo:hi, :])

            def load_xq(q):
                sl = slice(q * Q, (q + 1) * Q)
                nc.gpsimd.dma_start(
                    out=xbig[:, NT - 2, sl], in_=x_r[:, NT - 2, sl]
                )

            gather(0)
            load_xf(0, 2)
            gather(1)
            load_xf(2, 4)
            gather(2)
            load_xf(4, 6)
            gather(3)
            gather(7)
            gather(4)
            gather(5)
            gather(6)
            for q in range(NQ):
                load_xq(q)

            # sub on DVE, then square + f32 accum. ACT is capped at
            # 1 elem/cycle (~2.3us/tile + accumulator read), so two early
            # tiles square on DVE instead (mult + reduce_sum) to keep the
            # ACT chain off the critical path.
            SQ_ENGINE_FULL = {1: "dve", 2: "dve"}
            SQ_ENGINE_QUARTER = {}

            def square_accum(dif_ap, width, acc_col, eng):
                if eng == "act":
                    nc.scalar.activation(
                        out=dsq[:, 0:width],
                        in_=dif_ap,
                        func=mybir.ActivationFunctionType.Square,
                        accum_out=acc[:, acc_col:acc_col + 1],
                    )
                else:
                    nc.vector.tensor_mul(
                        out=tjunk[:, 0:width], in0=dif_ap, in1=dif_ap
                    )
                    nc.vector.reduce_sum(
                        out=acc[:, acc_col:acc_col + 1],
                        in_=tjunk[:, 0:width],
                        axis=mybir.AxisListType.X,
                    )

            # full tiles 0..5 and 7 (tile 7's pair lands mid-stream); acc
            # col 6 holds tile 7, cols 7..10 the quarters of tile 6.
            # DVE-squared tiles keep their dif in dedicated buffers and
            # their mult+reduce runs AFTER all subs, so the sub chain feeds
            # the ACT pipeline without 2.3us reduce stalls in the middle.
            deferred = []
            for i, n in enumerate((0, 1, 2, 3, 7, 4, 5)):
                col = n if n < NT - 2 else NT - 2
                if n in SQ_ENGINE_FULL:
                    dif = dif_c if n == 1 else dif_d
                else:
                    dif = dif_a if i % 2 == 0 else dif_b
                nc.vector.tensor_sub(
                    out=dif[:], in0=xbig[:, n, :], in1=ctile[:, n, :]
                )
                if n in SQ_ENGINE_FULL:
                    deferred.append((dif, col))
                else:
                    square_accum(dif[:], D, col, "act")
            for q in range(NQ):
                sl = slice(q * Q, (q + 1) * Q)
                dif = dif_b if q % 2 == 0 else dif_a
                nc.vector.tensor_sub(
                    out=dif[:, sl], in0=xbig[:, NT - 2, sl], in1=ctile[:, NT - 2, sl]
                )
                square_accum(
                    dif[:, sl], Q, NT - 1 + q, SQ_ENGINE_QUARTER.get(q, "act")
                )
            for dif, col in deferred:
                square_accum(dif[:], D, col, "dve")

            # out store on the warmed sync ring (the idle Sync engine issues
            # it as soon as the last accumulator lands; the SWDGE drain does
            # not have to wait for it)
            nc.sync.dma_start(out=out[:], in_=acc[:])

    nc.compile()
    return nc


_NC = None


def _get_nc():
    global _NC
    if _NC is None:
        _NC = build_nc()
    return _NC


def _shard(inputs: dict):
    x = np.asarray(inputs["x"]).astype(NP_BF16)
    labels = np.asarray(inputs["labels"]).astype(NP_BF16)
    centers = np.ascontiguousarray(np.asarray(inputs["centers"]).astype(NP_BF16))
    assert x.shape == (B, D) and labels.shape == (B, C) and centers.shape == (C, D)
    return [
        {
            "x": np.ascontiguousarray(x[k * BS:(k + 1) * BS]),
            "labels": np.ascontiguousarray(labels[k * BS:(k + 1) * BS]),
            "centers": centers,
        }
        for k in range(NCORES)
    ]


def run_sharded(inputs: dict, trace: bool = False):
    """Shard, run on 8 cores, return (per_sample [B] f32, BassKernelResults)."""
    in_maps = _shard(inputs)
    res = run_bass_kernel_spmd(
        _get_nc(), in_maps, core_ids=list(range(NCORES)), trace=trace
    )
    # out[p, n] holds sample k*BS + n*P + p; cols NT-1.. are the NQ
    # quarter-sums of the last tile
    def merge(o):
        # cols 0..5 = tiles 0..5, col 6 = tile 7, cols 7.. = tile 6 quarters
        t6 = o[:, NT - 1:].sum(axis=1, keepdims=True)
        return np.concatenate([o[:, :NT - 2], t6, o[:, NT - 2:NT - 1]], axis=1)

    per_sample = np.concatenate(
        [merge(res.results[k]["out"]).T.reshape(-1) for k in range(NCORES)]
    )
    return per_sample, res


def kernel(x, labels, centers):
    per_sample, _ = run_sharded({"x": x, "labels": labels, "centers": centers})
    per_sample = np.clip(per_sample, CLIP_LO, CLIP_HI)
    return np.asarray(per_sample.mean(dtype=np.float64), dtype=np.float32)
